# revision 1
# baseline (speedup 1.0000x reference)
"""Depthwise 1d (per-channel linear) Trainium2 Bass kernel.

out[n, c, o] = sum_i x[n, c, i] * W[c, o, i] + b[c, o]
  x: [4096, 256, 64] f32, W: [256, 128, 64] f32, b: [256, 128] f32
  out: [4096, 256, 128] f32

Strategy: pure data parallel over batch N across 8 cores (512 rows/core).
Per core, per channel c: out_c = x_c @ W_c.T  -- a [512,64]@[64,128] matmul.

The TensorE contracts over the partition dim, so x must be presented as
[i, n] per channel: we load x naturally ([n, (c,i)]), PE-transpose 128x128
chunks (2 channels each) giving lhsT = [(2ch x 64i), n].  Each channel
pair is then computed with base-partition-0 matmuls against a
block-diagonal rhs [128, 256] (upper-left = W_c0.T, lower-right = W_c1.T)
-- this keeps the full 128-row contraction busy without tile_position row
packing (which faults on this toolchain for fp32).

fp32 matmuls run at 1/4 rate on the PE, so the product is computed in
near-fp32 precision as three full-rate fp16 matmuls accumulated in fp32
PSUM:  x*W ~= xh*Wh + xh*Wl + xl*Wh  where xh = fp16(x), xl = fp16(x-xh)
(relative error ~1e-6, W split precomputed on the host).  The x hi/lo
split is fused into the PSUM evacuation of the fp32 transposes: ScalarE
casts hi, VectorE subtracts lo.

The PE stream is software-pipelined one iteration deep -- iteration k+1's
transposes are emitted before iteration k's matmuls -- so the VectorE
subtraction feeding the xl matmul pass has a whole matmul phase of slack
and the PE never stalls on it.  Block-diagonal weights are assembled on
device into pre-zeroed ping-pong tiles with two half DMAs per channel
block, prefetched one block ahead on the SWDGE ring; only the compact
~4.2 MB of fp16 weights crosses HBM.  Bias is added on the DVE during
PSUM evacuation of the matmul results, using a GPSIMD partition-broadcast
bias tile per channel block.  Output stores ride the ACT HWDGE ring so x
loads never queue behind them.
"""

import os

# recover cleanly if a previous run left the NeuronCores wedged; must be
# set before the runtime initializes
os.environ.setdefault("NEURON_RT_RESET_CORES", "1")

import numpy as np

import concourse.bass as bass
import concourse.tile as tile
from concourse import bacc, mybir
from concourse.bass_utils import run_bass_kernel_spmd

N_CORES = 8
N, C, HI, HO = 4096, 256, 64, 128
NLOC = N // N_CORES  # 512 batch rows per core
CB = 32              # channels per block
NT = 128             # batch rows per tile

F32 = mybir.dt.float32
F16 = mybir.dt.float16


def build(nloc=NLOC, c=C, cb=CB, n_cores=N_CORES):
    nc = bacc.Bacc(
        "TRN2", target_bir_lowering=False, debug=False, num_devices=n_cores
    )
    x_d = nc.dram_tensor("x", [nloc, c, HI], F32, kind="ExternalInput").ap()
    # compact packed hi/lo weights: [k, pair, 2, o]; k<64 -> ch 2j, k>=64 -> ch 2j+1
    w_d = nc.dram_tensor("w2", [128, c // 2, 2, HO], F16, kind="ExternalInput").ap()
    b_d = nc.dram_tensor("bias", [c, HO], F32, kind="ExternalInput").ap()
    i_d = nc.dram_tensor("ident", [128, 128], F32, kind="ExternalInput").ap()
    o_d = nc.dram_tensor("out", [nloc, c, HO], F32, kind="ExternalOutput").ap()

    n_tiles = nloc // NT
    c_blocks = c // cb
    pairs = cb // 2  # channel pairs per block

    with tile.TileContext(nc) as tc:
        with (
            tc.tile_pool(name="const", bufs=1) as const,
            tc.tile_pool(name="xp", bufs=3) as xp,
            tc.tile_pool(name="xhp", bufs=3) as xhp,
            tc.tile_pool(name="xlp", bufs=3) as xlp,
            tc.tile_pool(name="op", bufs=3) as op,
            tc.tile_pool(name="bp1", bufs=2) as bp1,
            tc.tile_pool(name="bp", bufs=2) as bp,
            tc.tile_pool(name="pst", bufs=3, space="PSUM") as pst,
            tc.tile_pool(name="pso", bufs=5, space="PSUM") as pso,
        ):
            ident = const.tile([128, 128], F32)
            nc.sync.dma_start(out=ident, in_=i_d)

            # persistent block-diagonal weight tiles (ping-pong across
            # channel blocks); zero once, then only the diagonal blocks
            # are rewritten by DMA each block.  dim 2 selects hi/lo.
            wtbd = [
                const.tile(
                    [128, pairs, 2, 2 * HO], F16, tag=f"wtbd{i}", name=f"wtbd{i}"
                )
                for i in range(2)
            ]
            for t in wtbd:
                nc.gpsimd.memset(t, 0.0)

            def load_block(ci):
                # SWDGE (gpsimd) ring so these strided loads never queue
                # ahead of the bulk x loads on the sync HWDGE ring
                c0 = ci * cb
                p0 = c0 // 2
                wt = wtbd[ci % 2]
                nc.gpsimd.dma_start(
                    out=wt[0:64, :, :, 0:HO], in_=w_d[0:64, p0 : p0 + pairs, :, :]
                )
                nc.gpsimd.dma_start(
                    out=wt[64:128, :, :, HO : 2 * HO],
                    in_=w_d[64:128, p0 : p0 + pairs, :, :],
                )
                b_one = bp1.tile([1, cb, HO], F32, name=f"b_one{ci}", tag="b_one")
                nc.gpsimd.dma_start(out=b_one, in_=b_d[c0 : c0 + cb, :])
                bias_sb = bp.tile(
                    [128, cb, HO], F32, name=f"bias_sb{ci}", tag="bias_sb"
                )
                nc.gpsimd.partition_broadcast(bias_sb, b_one)
                return wt, bias_sb

            def emit_T(ci, ni, blk):
                # x load, fp32 transposes, fused hi/lo fp16 split on
                # PSUM evacuation (ACT casts hi, DVE subtracts lo)
                c0 = ci * cb
                n0 = ni * NT
                x_sb = xp.tile([128, cb, HI], F32, name=f"x{ci}_{ni}", tag="x")
                nc.sync.dma_start(out=x_sb, in_=x_d[n0 : n0 + NT, c0 : c0 + cb, :])
                xh_sb = xhp.tile(
                    [128, pairs, NT], F16, name=f"xh{ci}_{ni}", tag="xh"
                )
                xl_sb = xlp.tile(
                    [128, pairs, NT], F16, name=f"xl{ci}_{ni}", tag="xl"
                )
                for g in range(pairs // 4):  # 4 fp32 pairs per PSUM bank
                    ps = pst.tile([128, 4, NT], F32)
                    for q in range(4):
                        j = g * 4 + q
                        nc.tensor.transpose(
                            ps[:, q, :], x_sb[:, 2 * j : 2 * j + 2, :], ident
                        )
                    sl = slice(g * 4, (g + 1) * 4)
                    nc.scalar.copy(out=xh_sb[:, sl, :], in_=ps)
                    nc.vector.tensor_sub(xl_sb[:, sl, :], ps, xh_sb[:, sl, :])
                return xh_sb, xl_sb, blk

            def emit_M(ci, ni, staged):
                xh_sb, xl_sb, (wt, bias_sb) = staged
                c0 = ci * cb
                n0 = ni * NT
                o_sb = op.tile([128, cb, HO], F32)
                for g in range(cb // 4):  # 4 channels / 2 pairs per bank
                    po = pso.tile([128, 4, HO], F32)
                    for p in range(2):
                        j = g * 2 + p  # pair within block
                        out_ap = po[:, 2 * p : 2 * p + 2, :]
                        nc.tensor.matmul(
                            out_ap, lhsT=xh_sb[:, j, :], rhs=wt[:, j, 0, :],
                            start=True, stop=False,
                        )
                        nc.tensor.matmul(
                            out_ap, lhsT=xh_sb[:, j, :], rhs=wt[:, j, 1, :],
                            start=False, stop=False,
                        )
                        nc.tensor.matmul(
                            out_ap, lhsT=xl_sb[:, j, :], rhs=wt[:, j, 0, :],
                            start=False, stop=True,
                        )
                    nc.vector.tensor_add(
                        out=o_sb[:, g * 4 : (g + 1) * 4, :],
                        in0=po,
                        in1=bias_sb[:, g * 4 : (g + 1) * 4, :],
                    )
                nc.scalar.dma_start(
                    out=o_d[n0 : n0 + NT, c0 : c0 + cb, :], in_=o_sb
                )

            iters = [
                (ci, ni) for ci in range(c_blocks) for ni in range(n_tiles)
            ]
            blocks = {0: load_block(0)}
            staged = emit_T(*iters[0], blocks[0])
            for idx, (ci, ni) in enumerate(iters):
                if ni == 0 and ci + 1 < c_blocks:
                    blocks[ci + 1] = load_block(ci + 1)
                cur = staged
                # pipeline: next iteration's transposes go to the PE ahead
                # of this iteration's matmuls
                if idx + 1 < len(iters):
                    nci, nni = iters[idx + 1]
                    staged = emit_T(nci, nni, blocks[nci])
                emit_M(ci, ni, cur)
    nc.compile()
    return nc


def pack_w(W):
    """[C, HO, HI] -> [128, C//2, 2, HO] fp16 hi/lo channel-pair packing."""
    C_, HO_, HI_ = W.shape
    Wh = W.astype(np.float16)
    Wl = (W - Wh.astype(np.float32)).astype(np.float16)
    out = np.empty((128, C_ // 2, 2, HO_), dtype=np.float16)
    # [C, HO, HI] -> [HI, C_pair, HO]
    out[:64, :, 0] = Wh[0::2].transpose(2, 0, 1)
    out[:64, :, 1] = Wl[0::2].transpose(2, 0, 1)
    out[64:, :, 0] = Wh[1::2].transpose(2, 0, 1)
    out[64:, :, 1] = Wl[1::2].transpose(2, 0, 1)
    return np.ascontiguousarray(out)


_cache = {}


def kernel(x, W, b):
    nc = _cache.get("nc")
    if nc is None:
        nc = _cache["nc"] = build()
    xs = np.ascontiguousarray(np.asarray(x, dtype=np.float32))
    Wt = pack_w(np.asarray(W, dtype=np.float32))
    bb = np.ascontiguousarray(np.asarray(b, dtype=np.float32))
    ident = np.eye(128, dtype=np.float32)
    in_maps = [
        {"x": xs[i * NLOC : (i + 1) * NLOC], "w2": Wt, "bias": bb, "ident": ident}
        for i in range(N_CORES)
    ]
    res = run_bass_kernel_spmd(nc, in_maps, core_ids=list(range(N_CORES)))
    return np.concatenate(
        [res.results[i]["out"] for i in range(N_CORES)], axis=0
    )

